# revision 6
# baseline (speedup 1.0000x reference)
"""Trainium2 Bass kernel for nn_Attention_77446850281941.

Computes, for dec_hidden [32,1024], enc_outputs [2048,32,1024], W [1,2048], b [1]:
    e[b,s]  = dec_hidden[b]@W[0,:1024] + enc_outputs[s,b,:]@W[0,1024:] + b[0]
    out     = softmax(tanh(e), axis=s)            -> [32, 2048] float32

Sharding: batch (32) is split across 8 NeuronCores (4 rows each); W/b are
replicated. Softmax rows live entirely on one core, so no collectives.

Per-core layout: enc shard [2048, 4, 1024] is streamed in s-chunks of 128
(partition dim = s, free = (b, e), 16KB contiguous per partition per DMA).
VectorE fused tensor_tensor_reduce does (enc*w_enc) + free-axis sum in one
pass; ScalarE applies tanh (folding the per-b dec contribution via the
per-partition bias port) and exp; tanh outputs are in [-1,1] so softmax
needs no max subtraction.  Row sums cross partitions via a PE ones-matmul,
and the final [128, 64] tile is PE-transposed so the output DMA writes
contiguous 512B rows.
"""

import sys

import numpy as np

for _p in ("/opt/trn_rl_repo",):
    if _p not in sys.path:
        sys.path.insert(0, _p)

import concourse.bacc as bacc
import concourse.tile as tile
from concourse import mybir
from concourse.bass_utils import run_bass_kernel_spmd

F32 = mybir.dt.float32
SRC = 2048          # src_len
BATCH = 32
EH2 = 1024          # 2*enc_hid_dim
DH = 1024           # dec_hid_dim
NCORES = 8
BPC = BATCH // NCORES      # batch rows per core = 4
NCHUNK = SRC // 128        # s-chunks per core = 16
SLAB_BUFS = 6
# "stt": one fused VectorE scalar_tensor_tensor (mult + free-axis accum).
# "act": VectorE tensor_mul then ScalarE Identity-activation with accum_out.
# "ttr": VectorE tensor_tensor_reduce (custom DVE uop; not on this runtime).
REDUCE_MODE = "stt"

_NC_CACHE = {}


def build_nc():
    nc = bacc.Bacc("TRN2", target_bir_lowering=False, debug=False)

    enc = nc.dram_tensor("enc", [SRC, BPC, EH2], F32, kind="ExternalInput").ap()
    dec = nc.dram_tensor("dec", [BPC, DH], F32, kind="ExternalInput").ap()
    w_enc = nc.dram_tensor("w_enc_bc", [128, EH2], F32, kind="ExternalInput").ap()
    w_dec = nc.dram_tensor("w_dec_bc", [BPC, DH], F32, kind="ExternalInput").ap()
    bias = nc.dram_tensor("bias_bc", [BPC, 1], F32, kind="ExternalInput").ap()
    # bias folded into dec_contrib via the reduce, or added post-hoc
    ones_col = nc.dram_tensor("ones_col", [128, 1], F32, kind="ExternalInput").ap()
    ones_row = nc.dram_tensor("ones_row", [1, 128], F32, kind="ExternalInput").ap()
    ident = nc.dram_tensor("ident", [128, 128], F32, kind="ExternalInput").ap()
    out = nc.dram_tensor("out", [BPC * NCHUNK, 128], F32, kind="ExternalOutput").ap()

    MUL = mybir.AluOpType.mult
    ADD = mybir.AluOpType.add
    ACT = mybir.ActivationFunctionType

    with tile.TileContext(nc) as tc:
        with (
            tc.tile_pool(name="consts", bufs=1) as consts,
            tc.tile_pool(name="slabs", bufs=SLAB_BUFS) as slabs,
            tc.tile_pool(name="scratch", bufs=2) as scratch,
            tc.tile_pool(name="acc", bufs=1) as acc,
            tc.tile_pool(name="small", bufs=1) as small,
            tc.tile_pool(name="psum", bufs=1, space="PSUM") as psum,
        ):
            w_sb = consts.tile([128, EH2], F32)
            nc.sync.dma_start(out=w_sb, in_=w_enc)
            dec_sb = consts.tile([BPC, DH], F32)
            nc.sync.dma_start(out=dec_sb, in_=dec)
            wdec_sb = consts.tile([BPC, DH], F32)
            nc.sync.dma_start(out=wdec_sb, in_=w_dec)
            bias_sb = consts.tile([BPC, 1], F32)
            nc.sync.dma_start(out=bias_sb, in_=bias)
            onec_sb = consts.tile([128, 1], F32)
            nc.sync.dma_start(out=onec_sb, in_=ones_col)
            oner_sb = consts.tile([1, 128], F32)
            nc.sync.dma_start(out=oner_sb, in_=ones_row)
            id_sb = consts.tile([128, 128], F32)
            nc.sync.dma_start(out=id_sb, in_=ident)

            # dec_contrib[b] = dec[b]·w_dec + bias, broadcast to [128, BPC]
            dec_scr = small.tile([BPC, DH], F32)
            dec_raw = small.tile([BPC, 1], F32)
            if REDUCE_MODE == "stt":
                nc.vector.scalar_tensor_tensor(
                    out=dec_scr, in0=dec_sb, scalar=1.0, in1=wdec_sb,
                    op0=MUL, op1=MUL, accum_out=dec_raw)
            elif REDUCE_MODE == "act":
                nc.vector.tensor_mul(dec_scr, dec_sb, wdec_sb)
                nc.scalar.activation(
                    out=dec_scr, in_=dec_scr,
                    func=mybir.ActivationFunctionType.Identity,
                    accum_out=dec_raw)
            else:
                nc.vector.tensor_tensor_reduce(
                    out=dec_scr, in0=dec_sb, in1=wdec_sb, scale=1.0,
                    scalar=0.0, op0=MUL, op1=ADD, accum_out=dec_raw)
            dec_c = small.tile([BPC, 1], F32)
            nc.vector.tensor_add(dec_c, dec_raw, bias_sb)
            p_row = psum.tile([1, BPC], F32)
            nc.tensor.transpose(p_row, dec_c, id_sb[0:BPC, 0:BPC])
            row_sb = small.tile([1, BPC], F32)
            nc.vector.tensor_copy(row_sb, p_row)
            p_bc = psum.tile([128, BPC], F32)
            nc.tensor.matmul(p_bc, oner_sb, row_sb)
            dec_bc = small.tile([128, BPC], F32)
            nc.vector.tensor_copy(dec_bc, p_bc)

            # e_cols[p, b, t] = enc[t*128+p, b, :]·w_enc
            e_cols = acc.tile([128, BPC, NCHUNK], F32)
            for t in range(NCHUNK):
                slab = slabs.tile([128, BPC, EH2], F32)
                nc.sync.dma_start(out=slab, in_=enc[t * 128:(t + 1) * 128, :, :])
                for b_ in range(BPC):
                    scr = scratch.tile([128, EH2], F32)
                    e_col = e_cols[:, b_, t:t + 1]
                    if REDUCE_MODE == "stt":
                        nc.vector.scalar_tensor_tensor(
                            out=scr, in0=slab[:, b_, :], scalar=1.0, in1=w_sb,
                            op0=MUL, op1=MUL, accum_out=e_col)
                    elif REDUCE_MODE == "act":
                        nc.vector.tensor_mul(scr, slab[:, b_, :], w_sb)
                        nc.scalar.activation(
                            out=scr, in_=scr,
                            func=mybir.ActivationFunctionType.Identity,
                            accum_out=e_col)
                    else:
                        nc.vector.tensor_tensor_reduce(
                            out=scr, in0=slab[:, b_, :], in1=w_sb, scale=1.0,
                            scalar=0.0, op0=MUL, op1=ADD, accum_out=e_col)

            # tanh(e + dec_contrib[b]) then exp; tanh in [-1,1] -> stable softmax
            texp = acc.tile([128, BPC, NCHUNK], F32)
            for b_ in range(BPC):
                nc.scalar.activation(
                    out=texp[:, b_, :], in_=e_cols[:, b_, :],
                    func=ACT.Tanh, bias=dec_bc[:, b_:b_ + 1], scale=1.0)
            exp_t = acc.tile([128, BPC, NCHUNK], F32)
            nc.scalar.activation(out=exp_t[:, :, :], in_=texp[:, :, :], func=ACT.Exp)

            # denominator: per-b sum over free (t) then over partitions (s)
            sums = small.tile([128, BPC], F32)
            nc.vector.tensor_reduce(
                out=sums, in_=exp_t[:, :, :],
                axis=mybir.AxisListType.X, op=ADD)
            p_tot = psum.tile([1, BPC], F32)
            nc.tensor.matmul(p_tot, onec_sb, sums)
            tot_sb = small.tile([1, BPC], F32)
            nc.vector.tensor_copy(tot_sb, p_tot)
            rec_sb = small.tile([1, BPC], F32)
            nc.vector.reciprocal(rec_sb, tot_sb)
            p_rbc = psum.tile([128, BPC], F32)
            nc.tensor.matmul(p_rbc, oner_sb, rec_sb)
            rec_bc = small.tile([128, BPC], F32)
            nc.vector.tensor_copy(rec_bc, p_rbc)

            attn = acc.tile([128, BPC, NCHUNK], F32)
            for b_ in range(BPC):
                nc.vector.tensor_scalar_mul(
                    out=attn[:, b_, :], in0=exp_t[:, b_, :],
                    scalar1=rec_bc[:, b_:b_ + 1])

            # [128, (b,t)] -> [(b,t), 128] so output rows are contiguous
            p_out = psum.tile([BPC * NCHUNK, 128], F32)
            nc.tensor.transpose(p_out, attn[:, :, :], id_sb)
            out_sb = small.tile([BPC * NCHUNK, 128], F32)
            nc.vector.tensor_copy(out_sb, p_out)
            nc.sync.dma_start(out=out, in_=out_sb)

    nc.finalize()
    return nc


def _get_nc():
    if "nc" not in _NC_CACHE:
        _NC_CACHE["nc"] = build_nc()
    return _NC_CACHE["nc"]


def make_in_maps(dec_hidden, enc_outputs, W, b):
    f32 = np.float32
    w_enc_bc = np.ascontiguousarray(
        np.broadcast_to(W[0, DH:].astype(f32), (128, EH2)))
    w_dec_bc = np.ascontiguousarray(
        np.broadcast_to(W[0, :DH].astype(f32), (BPC, DH)))
    bias_bc = np.full((BPC, 1), np.float32(b[0]), dtype=f32)
    ones_col = np.ones((128, 1), dtype=f32)
    ones_row = np.ones((1, 128), dtype=f32)
    ident = np.eye(128, dtype=f32)
    in_maps = []
    for i in range(NCORES):
        in_maps.append({
            "enc": np.ascontiguousarray(
                enc_outputs[:, i * BPC:(i + 1) * BPC, :].astype(f32)),
            "dec": np.ascontiguousarray(
                dec_hidden[i * BPC:(i + 1) * BPC, :].astype(f32)),
            "w_enc_bc": w_enc_bc,
            "w_dec_bc": w_dec_bc,
            "bias_bc": bias_bc,
            "ones_col": ones_col,
            "ones_row": ones_row,
            "ident": ident,
        })
    return in_maps


def assemble_output(results):
    return np.concatenate(
        [r["out"].reshape(BPC, SRC) for r in results], axis=0).astype(np.float32)


def kernel(dec_hidden, enc_outputs, W, b):
    nc = _get_nc()
    in_maps = make_in_maps(dec_hidden, enc_outputs, W, b)
    res = run_bass_kernel_spmd(nc, in_maps, core_ids=list(range(NCORES)))
    return assemble_output(res.results)


# revision 7
# speedup vs baseline: 1.0216x; 1.0216x over previous
"""Trainium2 Bass kernel for nn_Attention_77446850281941.

Computes, for dec_hidden [32,1024], enc_outputs [2048,32,1024], W [1,2048], b [1]:
    e[b,s]  = dec_hidden[b]@W[0,:1024] + enc_outputs[s,b,:]@W[0,1024:] + b[0]
    out     = softmax(tanh(e), axis=s)            -> [32, 2048] float32

Sharding: batch (32) is split across 8 NeuronCores (4 rows each); W/b are
replicated. Softmax rows live entirely on one core, so no collectives.

Per-core dataflow (DMA-bound at ~358 GB/s; 32 MB of enc per core):
 - enc shard [2048, 4, 1024] streams in s-chunks of 128 (partition = s,
   free = (b, e); 16 KB contiguous per partition per DMA).
 - VectorE scalar_tensor_tensor fuses (enc * w_enc) with the free-axis sum
   in a single pass per (chunk, b); full multiply result is dumped to a
   stride-0 scratch column (only the accumulator matters).
 - ScalarE applies tanh (folding the per-b dec_hidden·w_dec + bias via the
   per-partition bias port) and exp per chunk as columns arrive, so the
   post-loop work is just the softmax normalization. tanh output is in
   [-1,1], so exp needs no max subtraction.
 - Row sums cross partitions via a PE ones-matmul; the final [128, 64]
   tile is PE-transposed so the output DMA writes contiguous 512B rows.
"""

import sys

import numpy as np

for _p in ("/opt/trn_rl_repo",):
    if _p not in sys.path:
        sys.path.insert(0, _p)

import concourse.bacc as bacc
import concourse.tile as tile
from concourse import mybir
from concourse.bass_utils import run_bass_kernel_spmd

F32 = mybir.dt.float32
SRC = 2048          # src_len
BATCH = 32
EH2 = 1024          # 2*enc_hid_dim
DH = 1024           # dec_hid_dim
NCORES = 8
BPC = BATCH // NCORES      # batch rows per core = 4
NCHUNK = SRC // 128        # s-chunks per core = 16
SLAB_BUFS = 6
SPLIT_FIRST = 1            # how many leading slabs get per-b sub-DMAs

_NC_CACHE = {}


def build_nc():
    nc = bacc.Bacc("TRN2", target_bir_lowering=False, debug=False)

    enc = nc.dram_tensor("enc", [SRC, BPC, EH2], F32, kind="ExternalInput").ap()
    # dec row, w_dec row, bias packed host-side: [BPC, 2*DH + 1]
    dpack = nc.dram_tensor("dpack", [BPC, 2 * DH + 1], F32,
                           kind="ExternalInput").ap()
    w_enc = nc.dram_tensor("w_enc_bc", [128, EH2], F32, kind="ExternalInput").ap()
    ident = nc.dram_tensor("ident", [128, 128], F32, kind="ExternalInput").ap()
    out = nc.dram_tensor("out", [BPC * NCHUNK, 128], F32, kind="ExternalOutput").ap()

    MUL = mybir.AluOpType.mult
    ADD = mybir.AluOpType.add
    ACT = mybir.ActivationFunctionType

    with tile.TileContext(nc) as tc:
        with (
            tc.tile_pool(name="consts", bufs=1) as consts,
            tc.tile_pool(name="slabs", bufs=SLAB_BUFS) as slabs,
            tc.tile_pool(name="firsts", bufs=BPC * SPLIT_FIRST) as firsts,
            tc.tile_pool(name="acc", bufs=1) as acc,
            tc.tile_pool(name="small", bufs=1) as small,
            tc.tile_pool(name="psum", bufs=1, space="PSUM") as psum,
        ):
            # main-loop weights first on the sync HWDGE ring, then the slabs
            w_sb = consts.tile([128, EH2], F32)
            nc.sync.dma_start(out=w_sb, in_=w_enc)
            # small consts ride the scalar HWDGE ring to keep sync free
            dp_sb = consts.tile([BPC, 2 * DH + 1], F32)
            nc.scalar.dma_start(out=dp_sb, in_=dpack)
            id_sb = consts.tile([128, 128], F32)
            nc.scalar.dma_start(out=id_sb, in_=ident)
            onec_sb = consts.tile([128, 1], F32)
            nc.gpsimd.memset(onec_sb, 1.0)
            oner_sb = consts.tile([1, 128], F32)
            nc.gpsimd.memset(oner_sb, 1.0)

            # stride-0 dump column for the unused full multiply result
            dump = small.tile([128, 1], F32)

            # dec_contrib[b] = dec[b]·w_dec + bias, broadcast to [128, BPC]
            dec_c = small.tile([BPC, 1], F32)
            nc.vector.scalar_tensor_tensor(
                out=dump[:BPC, :].broadcast_to((BPC, DH)),
                in0=dp_sb[:, 0:DH], scalar=1.0, in1=dp_sb[:, DH:2 * DH],
                op0=MUL, op1=MUL, accum_out=dec_c)
            dec_cb = small.tile([BPC, 1], F32)
            nc.vector.tensor_add(dec_cb, dec_c, dp_sb[:, 2 * DH:2 * DH + 1])
            p_row = psum.tile([1, BPC], F32)
            nc.tensor.transpose(p_row, dec_cb, id_sb[0:BPC, 0:BPC])
            row_sb = small.tile([1, BPC], F32)
            nc.vector.tensor_copy(row_sb, p_row)
            p_bc = psum.tile([128, BPC], F32)
            nc.tensor.matmul(p_bc, oner_sb, row_sb)
            dec_bc = small.tile([128, BPC], F32)
            nc.vector.tensor_copy(dec_bc, p_bc)

            # e_cols[p, b, t] = enc[t*128+p, b, :]·w_enc;  texp = exp(tanh(...))
            e_cols = acc.tile([128, BPC, NCHUNK], F32)
            texp = acc.tile([128, BPC, NCHUNK], F32)
            exp_t = acc.tile([128, BPC, NCHUNK], F32)
            for t in range(NCHUNK):
                if t < SPLIT_FIRST:
                    # split the first slab(s) so VectorE starts after 512 KB
                    parts = []
                    for b_ in range(BPC):
                        sub = firsts.tile([128, EH2], F32, tag="first")
                        nc.sync.dma_start(
                            out=sub, in_=enc[t * 128:(t + 1) * 128, b_, :])
                        parts.append(sub)
                    bslice = lambda b_: parts[b_]
                else:
                    slab = slabs.tile([128, BPC, EH2], F32)
                    nc.sync.dma_start(
                        out=slab, in_=enc[t * 128:(t + 1) * 128, :, :])
                    bslice = lambda b_: slab[:, b_, :]
                for b_ in range(BPC):
                    nc.vector.scalar_tensor_tensor(
                        out=dump.broadcast_to((128, EH2)),
                        in0=bslice(b_), scalar=1.0, in1=w_sb,
                        op0=MUL, op1=MUL,
                        accum_out=e_cols[:, b_, t:t + 1])
                    nc.scalar.activation(
                        out=texp[:, b_, t:t + 1], in_=e_cols[:, b_, t:t + 1],
                        func=ACT.Tanh, bias=dec_bc[:, b_:b_ + 1], scale=1.0)
                nc.scalar.activation(
                    out=exp_t[:, :, t:t + 1], in_=texp[:, :, t:t + 1],
                    func=ACT.Exp)

            # softmax denominator: per-b sum over t (free) then s (partitions)
            sums = small.tile([128, BPC], F32)
            nc.vector.tensor_reduce(
                out=sums, in_=exp_t[:, :, :],
                axis=mybir.AxisListType.X, op=ADD)
            p_tot = psum.tile([1, BPC], F32)
            nc.tensor.matmul(p_tot, onec_sb, sums)
            tot_sb = small.tile([1, BPC], F32)
            nc.vector.tensor_copy(tot_sb, p_tot)
            rec_sb = small.tile([1, BPC], F32)
            nc.vector.reciprocal(rec_sb, tot_sb)
            p_rbc = psum.tile([128, BPC], F32)
            nc.tensor.matmul(p_rbc, oner_sb, rec_sb)
            rec_bc = small.tile([128, BPC], F32)
            nc.vector.tensor_copy(rec_bc, p_rbc)

            attn = acc.tile([128, BPC, NCHUNK], F32)
            for b_ in range(BPC):
                nc.vector.tensor_scalar_mul(
                    out=attn[:, b_, :], in0=exp_t[:, b_, :],
                    scalar1=rec_bc[:, b_:b_ + 1])

            # [128, (b,t)] -> [(b,t), 128] so output rows are contiguous
            p_out = psum.tile([BPC * NCHUNK, 128], F32)
            nc.tensor.transpose(p_out, attn[:, :, :], id_sb)
            out_sb = small.tile([BPC * NCHUNK, 128], F32)
            nc.vector.tensor_copy(out_sb, p_out)
            nc.sync.dma_start(out=out, in_=out_sb)

    nc.finalize()
    return nc


def _get_nc():
    if "nc" not in _NC_CACHE:
        _NC_CACHE["nc"] = build_nc()
    return _NC_CACHE["nc"]


def make_in_maps(dec_hidden, enc_outputs, W, b):
    f32 = np.float32
    w_enc_bc = np.ascontiguousarray(
        np.broadcast_to(W[0, DH:].astype(f32), (128, EH2)))
    ident = np.eye(128, dtype=f32)
    w_dec = np.asarray(W[0, :DH], dtype=f32)
    bias = np.float32(b[0])
    in_maps = []
    for i in range(NCORES):
        dec_i = np.asarray(dec_hidden[i * BPC:(i + 1) * BPC, :], dtype=f32)
        dpack = np.concatenate(
            [dec_i,
             np.broadcast_to(w_dec, (BPC, DH)),
             np.full((BPC, 1), bias, dtype=f32)], axis=1)
        in_maps.append({
            "enc": np.ascontiguousarray(
                enc_outputs[:, i * BPC:(i + 1) * BPC, :].astype(f32)),
            "dpack": np.ascontiguousarray(dpack),
            "w_enc_bc": w_enc_bc,
            "ident": ident,
        })
    return in_maps


def assemble_output(results):
    return np.concatenate(
        [r["out"].reshape(BPC, SRC) for r in results], axis=0).astype(np.float32)


def kernel(dec_hidden, enc_outputs, W, b):
    nc = _get_nc()
    in_maps = make_in_maps(dec_hidden, enc_outputs, W, b)
    res = run_bass_kernel_spmd(nc, in_maps, core_ids=list(range(NCORES)))
    return assemble_output(res.results)
